# revision 12
# baseline (speedup 1.0000x reference)
"""Trainium2 Bass kernel for ConnectedFilterLayerWithImplicitJacobian.

Host marshalling orders the 2N (node, +/-) pairs by Euler-tour time t, so the
delta array IS phase A's output in layout order — no scatter, no transpose,
no collectives:

  A. stream filtered = sign * residues * sigmoid(attrs @ w + b) for all 2N
     t-ordered rows directly into the scan tile (replicated on all 8 cores)
  D. prefix-sum (shift-add cascade on DVE + cross-partition fix via PE)
  F. y[p] = cums[tpre[node_of_pixel[p]]] (host-composed index), sharded 1/8:
     128 batched element-wise indirect gathers (8192 descriptors each)
"""
import contextlib
import ctypes
import os
import sys
import types

sys.path.insert(0, "/opt/trn_rl_repo")

import numpy as np

# ---------------------------------------------------------------- shims ----
_SO_PATH = "/opt/axon/libaxon_pjrt.so"


def _install_ntff_shim():
    if "antenv.axon_hooks" in sys.modules:
        return
    try:
        lib = ctypes.CDLL(_SO_PATH)
        ok = hasattr(lib, "axon_start_nrt_profile")
    except OSError:
        ok = False
    if ok:
        lib.axon_start_nrt_profile.argtypes = [ctypes.POINTER(ctypes.c_int64), ctypes.c_size_t]
        lib.axon_start_nrt_profile.restype = ctypes.c_int64
        lib.axon_stop_nrt_profile.argtypes = [ctypes.c_char_p]
        lib.axon_stop_nrt_profile.restype = ctypes.c_int64

        @contextlib.contextmanager
        def _hook(output_dir, device_ids):
            import jax

            jax.devices()
            if device_ids:
                ids = (ctypes.c_int64 * len(device_ids))(*device_ids)
                rc = lib.axon_start_nrt_profile(ids, len(device_ids))
            else:
                rc = lib.axon_start_nrt_profile(None, 0)
            if rc != 0:
                raise RuntimeError(f"axon_start_nrt_profile rc={rc}")
            try:
                yield
            finally:
                n = lib.axon_stop_nrt_profile(str(output_dir).encode())
                if n < 0:
                    raise RuntimeError(f"axon_stop_nrt_profile rc={n}")
    else:
        _hook = None
    mod = types.ModuleType("antenv.axon_hooks")
    mod.get_axon_ntff_profile_hook = lambda: _hook
    mod.set_axon_ntff_profile_hook = lambda h: None
    sys.modules["antenv.axon_hooks"] = mod


_install_ntff_shim()

import concourse.bass as bass
import concourse.bass_utils as bass_utils
import concourse.mybir as mybir
import concourse.tile as tile
from concourse.bass_utils import run_bass_kernel_spmd

# walrus birsim on a large program is prohibitively slow; turn it off
_orig_run_command = bass_utils.run_command


def _patched_run_command(argv, **kwargs):
    argv = ["--enable-birsim=false" if a == "--enable-birsim=true" else a for a in argv]
    return _orig_run_command(argv, **kwargs)


bass_utils.run_command = _patched_run_command

MAX_WAITS = 1


def _split_excess_waits(nc):
    """This container's walrus accepts at most one sync-wait per instruction;
    move extra waits onto injected no-ops ahead of the instruction."""
    nid = 0
    for bb in nc.main_func.blocks:
        insts = bb.instructions
        targets = []
        for idx in range(len(insts)):
            ins = insts[idx]
            si = ins.sync_info
            if si is not None and si.on_wait is not None and len(si.on_wait) > MAX_WAITS:
                targets.append(ins.name)
        for name in targets:
            idx = next(i for i in range(len(insts)) if insts[i].name == name)
            ins = insts[idx]
            w = list(ins.sync_info.on_wait)
            excess, keep = w[:-MAX_WAITS], w[-MAX_WAITS:]
            ins.sync_info.on_wait = keep
            pos = idx
            while excess:
                chunk, excess = excess[:MAX_WAITS], excess[MAX_WAITS:]
                nop = mybir.InstNoOp(
                    name=f"I-ws-{nid}", engine=ins.engine, ins=[], outs=[],
                    sync_info=mybir.SyncInfo(on_wait=chunk, on_update=[]),
                )
                nid += 1
                insts.insert(pos, nop)
                pos += 1


# ------------------------------------------------------------- geometry ----
NCORES = 8
N = 500_000
K = 8
ROWS, COLS = 2048, 4096
P = ROWS * COLS                  # 8388608
PIXN = P // NCORES               # 1048576 pixels per core
T2N = 2 * N                      # 1000000
FD = 7816                        # 128*7816 = 1000448 >= 2N
DSZ = 128 * FD                   # 1000448
DPAD = 128 * 7817
SHIFTS = [1, 2, 4, 8, 16, 32, 64, 128, 256, 512, 1024, 2048, 4096]
MARG = 4096
TCH = 977                        # t-columns per phase-A stream chunk (8*977=FD)
CH = 8192                        # descriptors per batched indirect DMA
CHC = CH // 128                  # 64 offset columns per chunk
NCH_P = PIXN // CH               # 128 pixel chunks

_cache = {}
_last_res = [None]


def _build_program():
    if "nc" in _cache:
        return _cache["nc"]
    nc = bass.Bass()
    f32, i32 = mybir.dt.float32, mybir.dt.int32

    w_full = nc.dram_tensor("w_full", [128, TCH * K], f32, kind="ExternalInput")
    b_rep = nc.dram_tensor("b_rep", [128, 1], f32, kind="ExternalInput")
    ltm = nc.dram_tensor("ltm", [128, 128], f32, kind="ExternalInput")
    attrs_t = nc.dram_tensor("attrs_t", [128, FD * K], f32, kind="ExternalInput")
    res_t = nc.dram_tensor("res_t", [128, FD], f32, kind="ExternalInput")
    sgn_t = nc.dram_tensor("sgn_t", [128, FD], f32, kind="ExternalInput")
    pix_idx = nc.dram_tensor("pix_idx", [128, NCH_P * CHC], i32, kind="ExternalInput")
    y_sh = nc.dram_tensor("y_sh", [NCH_P, CH], f32, kind="ExternalOutput")

    cums_d = nc.dram_tensor("cums_d", [DPAD], f32)

    with tile.TileContext(nc) as tc:
        with tc.tile_pool(name="keep", bufs=1) as kp:
            lt_t = kp.tile([128, 128], f32)
            b_t = kp.tile([128, 1], f32)
            nc.sync.dma_start(lt_t[:], ltm[:])
            nc.sync.dma_start(b_t[:], b_rep[:])

            with tc.tile_pool(name="scan", bufs=1) as sp:
                wa = sp.tile([128, MARG + FD], f32)
                wb = sp.tile([128, MARG + FD], f32)
                off_sb = sp.tile([128, 1], f32)
                nc.vector.memset(wa[:], 0.0)
                nc.vector.memset(wb[:], 0.0)

                # ---- phase A: stream filtered into the scan tile ----
                with tc.tile_pool(name="pa", bufs=2) as pa:
                    w_t = pa.tile([128, TCH * K], f32)
                    nc.scalar.dma_start(w_t[:], w_full[:])
                    for c in range(8):
                        c0 = c * TCH
                        at = pa.tile([128, TCH * K], f32)
                        rt = pa.tile([128, TCH], f32)
                        sg = pa.tile([128, TCH], f32)
                        lg = pa.tile([128, TCH], f32)
                        nc.scalar.dma_start(at[:], attrs_t[:, c0 * K:(c0 + TCH) * K])
                        nc.scalar.dma_start(rt[:], res_t[:, c0:c0 + TCH])
                        nc.scalar.dma_start(sg[:], sgn_t[:, c0:c0 + TCH])
                        nc.vector.tensor_mul(at[:], at[:], w_t[:])
                        nc.vector.reduce_sum(
                            lg[:].rearrange("p (n o) -> p n o", o=1),
                            at[:].rearrange("p (n k) -> p n k", k=K),
                            axis=mybir.AxisListType.X,
                        )
                        nc.scalar.activation(
                            lg[:], lg[:], mybir.ActivationFunctionType.Sigmoid,
                            bias=b_t[:], scale=1.0,
                        )
                        nc.vector.tensor_mul(lg[:], lg[:], rt[:])
                        nc.vector.tensor_mul(
                            wa[:, MARG + c0:MARG + c0 + TCH], lg[:], sg[:]
                        )

                # ---- phase D: prefix sum ----
                cur, nxt = wa, wb
                for s in SHIFTS:
                    nc.vector.tensor_add(
                        nxt[:, MARG:], cur[:, MARG:], cur[:, MARG - s:MARG - s + FD]
                    )
                    cur, nxt = nxt, cur
                with tc.tile_pool(name="psc", bufs=1, space="PSUM") as pp:
                    ps = pp.tile([128, 1], f32, space="PSUM")
                    nc.tensor.matmul(
                        ps[:], lhsT=lt_t[:], rhs=cur[:, MARG + FD - 1:MARG + FD],
                        start=True, stop=True,
                    )
                    nc.vector.tensor_copy(off_sb[:], ps[:])
                nc.vector.tensor_scalar_add(cur[:, MARG:], cur[:, MARG:], off_sb[:])
                nc.sync.dma_start(
                    cums_d[0:DSZ].rearrange("(p f) -> p f", p=128), cur[:, MARG:]
                )

            # ---- phase F: pixel gather (element-wise batched, pipelined) ----
            cg = cums_d[:].rearrange("(n o) -> n o", o=1)
            with tc.tile_pool(name="pf", bufs=1) as pf:
                ix_t = pf.tile([128, NCH_P * CHC], i32)
                yt = pf.tile([128, CH], f32)
                nc.sync.dma_start(ix_t[:], pix_idx[:])
                for j in range(NCH_P):
                    row = j % 128
                    nc.gpsimd.indirect_dma_start(
                        out=yt[row:row + 1, :].rearrange("p (l u) -> p l u", u=1),
                        out_offset=None,
                        in_=cg,
                        in_offset=bass.IndirectOffsetOnAxis(
                            ap=ix_t[:, j * CHC:(j + 1) * CHC], axis=0),
                    )
                    nc.sync.dma_start(
                        y_sh[j:j + 1, :].rearrange("o (t l) -> o t l", t=2),
                        yt[row:row + 1, :].rearrange("p (t l) -> p t l", t=2),
                    )

    _split_excess_waits(nc)
    _cache["nc"] = nc
    return nc


def kernel(weight, bias, residues, attrs2d, tpre, tpost, node_of_pixel,
           numRows, numCols, _profile=[None]):
    weight = np.asarray(weight, np.float32)
    bias = np.asarray(bias, np.float32)
    residues = np.asarray(residues, np.float32)
    attrs2d = np.asarray(attrs2d, np.float32)
    tpre = np.asarray(tpre, np.int64)
    tpost = np.asarray(tpost, np.int64)
    nop = np.asarray(node_of_pixel, np.int64)
    numRows = int(numRows)
    numCols = int(numCols)

    # --- host-side marshalling: order (node, sign) pairs by Euler time t ---
    ordr = np.zeros(DSZ, np.int64)
    sgn = np.zeros(DSZ, np.float32)
    ar = np.arange(N)
    ordr[tpre] = ar
    sgn[tpre] = 1.0
    ordr[tpost] = ar
    sgn[tpost] = -1.0
    attrs_tt = attrs2d[ordr].astype(np.float32)      # (DSZ, K); pads -> row 0
    attrs_tt[sgn == 0.0] = 0.0
    res_tt = residues[ordr].astype(np.float32)
    res_tt[sgn == 0.0] = 0.0
    attrs_w = attrs_tt.reshape(128, FD * K)
    res_w = res_tt.reshape(128, FD)
    sgn_w = sgn.reshape(128, FD)

    w_full = np.tile(weight[None, :], (128, TCH)).astype(np.float32)
    b_rep = np.full((128, 1), np.float32(bias[0]), np.float32)
    lt = (np.arange(128)[:, None] < np.arange(128)[None, :]).astype(np.float32)

    # pixel gather index: t of the pixel's node, wrap transform for the
    # partition-fastest offset consumption order
    t_pix = tpre[nop].astype(np.int32)
    slot_sh = t_pix.reshape(NCORES, NCH_P, CHC, 128).transpose(0, 3, 1, 2) \
        .reshape(NCORES, 128, NCH_P * CHC)

    in_maps = []
    for r in range(NCORES):
        in_maps.append({
            "w_full": w_full,
            "b_rep": b_rep,
            "ltm": lt,
            "attrs_t": attrs_w,
            "res_t": res_w,
            "sgn_t": sgn_w,
            "pix_idx": np.ascontiguousarray(slot_sh[r]),
        })

    nc = _build_program()
    res = run_bass_kernel_spmd(nc, in_maps, list(range(NCORES)),
                               trace=bool(_profile[0]))
    _last_res[0] = res
    if _profile[0] is not None:
        _profile[0] = res.exec_time_ns

    y = np.concatenate([res.results[r]["y_sh"].reshape(-1) for r in range(NCORES)])
    return y.reshape(numRows, numCols).astype(np.float32)


# revision 13
# speedup vs baseline: 1.4296x; 1.4296x over previous
"""Trainium2 Bass kernel for ConnectedFilterLayerWithImplicitJacobian.

Host marshalling orders the 2N (node, +/-) pairs by Euler-tour time t, so the
delta array IS phase A's output in layout order — no scatter, no transpose,
no collectives:

  A. stream filtered = sign * residues * sigmoid(attrs @ w + b) for all 2N
     t-ordered rows directly into the scan tile (replicated on all 8 cores)
  D. prefix-sum (shift-add cascade on DVE + cross-partition fix via PE)
  F. y[p] = cums[tpre[node_of_pixel[p]]] (host-composed index), sharded 1/8:
     128 batched element-wise indirect gathers (8192 descriptors each)
"""
import contextlib
import ctypes
import os
import sys
import types

sys.path.insert(0, "/opt/trn_rl_repo")

import numpy as np

# ---------------------------------------------------------------- shims ----
_SO_PATH = "/opt/axon/libaxon_pjrt.so"


def _install_ntff_shim():
    if "antenv.axon_hooks" in sys.modules:
        return
    try:
        lib = ctypes.CDLL(_SO_PATH)
        ok = hasattr(lib, "axon_start_nrt_profile")
    except OSError:
        ok = False
    if ok:
        lib.axon_start_nrt_profile.argtypes = [ctypes.POINTER(ctypes.c_int64), ctypes.c_size_t]
        lib.axon_start_nrt_profile.restype = ctypes.c_int64
        lib.axon_stop_nrt_profile.argtypes = [ctypes.c_char_p]
        lib.axon_stop_nrt_profile.restype = ctypes.c_int64

        @contextlib.contextmanager
        def _hook(output_dir, device_ids):
            import jax

            jax.devices()
            if device_ids:
                ids = (ctypes.c_int64 * len(device_ids))(*device_ids)
                rc = lib.axon_start_nrt_profile(ids, len(device_ids))
            else:
                rc = lib.axon_start_nrt_profile(None, 0)
            if rc != 0:
                raise RuntimeError(f"axon_start_nrt_profile rc={rc}")
            try:
                yield
            finally:
                n = lib.axon_stop_nrt_profile(str(output_dir).encode())
                if n < 0:
                    raise RuntimeError(f"axon_stop_nrt_profile rc={n}")
    else:
        _hook = None
    mod = types.ModuleType("antenv.axon_hooks")
    mod.get_axon_ntff_profile_hook = lambda: _hook
    mod.set_axon_ntff_profile_hook = lambda h: None
    sys.modules["antenv.axon_hooks"] = mod


_install_ntff_shim()

import concourse.bass as bass
import concourse.bass_utils as bass_utils
import concourse.mybir as mybir
import concourse.tile as tile
from concourse.bass_utils import run_bass_kernel_spmd

# walrus birsim on a large program is prohibitively slow; turn it off
_orig_run_command = bass_utils.run_command


def _patched_run_command(argv, **kwargs):
    argv = ["--enable-birsim=false" if a == "--enable-birsim=true" else a for a in argv]
    return _orig_run_command(argv, **kwargs)


bass_utils.run_command = _patched_run_command

MAX_WAITS = 1


def _split_excess_waits(nc):
    """This container's walrus accepts at most one sync-wait per instruction;
    move extra waits onto injected no-ops ahead of the instruction."""
    nid = 0
    for bb in nc.main_func.blocks:
        insts = bb.instructions
        targets = []
        for idx in range(len(insts)):
            ins = insts[idx]
            si = ins.sync_info
            if si is not None and si.on_wait is not None and len(si.on_wait) > MAX_WAITS:
                targets.append(ins.name)
        for name in targets:
            idx = next(i for i in range(len(insts)) if insts[i].name == name)
            ins = insts[idx]
            w = list(ins.sync_info.on_wait)
            excess, keep = w[:-MAX_WAITS], w[-MAX_WAITS:]
            ins.sync_info.on_wait = keep
            pos = idx
            while excess:
                chunk, excess = excess[:MAX_WAITS], excess[MAX_WAITS:]
                nop = mybir.InstNoOp(
                    name=f"I-ws-{nid}", engine=ins.engine, ins=[], outs=[],
                    sync_info=mybir.SyncInfo(on_wait=chunk, on_update=[]),
                )
                nid += 1
                insts.insert(pos, nop)
                pos += 1


# ------------------------------------------------------------- geometry ----
NCORES = 8
N = 500_000
K = 8
ROWS, COLS = 2048, 4096
P = ROWS * COLS                  # 8388608
PIXN = P // NCORES               # 1048576 pixels per core
T2N = 2 * N                      # 1000000
FD = 7816                        # 128*7816 = 1000448 >= 2N
DSZ = 128 * FD                   # 1000448
DPAD = 128 * 7817
SHIFTS = [1, 2, 4, 8, 16, 32, 64, 128, 256, 512, 1024, 2048, 4096]
MARG = 4096
TCH = 977                        # t-columns per phase-A stream chunk (8*977=FD)
CH = 8192                        # descriptors per batched indirect DMA
CHC = CH // 128                  # 64 offset columns per chunk
NCH_P = PIXN // CH               # 128 pixel chunks

_cache = {}
_last_res = [None]


def _build_program():
    if "nc" in _cache:
        return _cache["nc"]
    nc = bass.Bass()
    f32, i32 = mybir.dt.float32, mybir.dt.int32

    w_full = nc.dram_tensor("w_full", [128, TCH * K], f32, kind="ExternalInput")
    b_rep = nc.dram_tensor("b_rep", [128, 1], f32, kind="ExternalInput")
    ltm = nc.dram_tensor("ltm", [128, 128], f32, kind="ExternalInput")
    attrs_t = nc.dram_tensor("attrs_t", [128, FD * K], f32, kind="ExternalInput")
    res_t = nc.dram_tensor("res_t", [128, FD], f32, kind="ExternalInput")
    sgn_t = nc.dram_tensor("sgn_t", [128, FD], f32, kind="ExternalInput")
    pix_idx = nc.dram_tensor("pix_idx", [128, NCH_P * CHC], i32, kind="ExternalInput")
    y_sh = nc.dram_tensor("y_sh", [NCH_P, CH], f32, kind="ExternalOutput")

    cums_d = nc.dram_tensor("cums_d", [DPAD], f32)

    with tile.TileContext(nc) as tc:
        with tc.tile_pool(name="keep", bufs=1) as kp:
            lt_t = kp.tile([128, 128], f32)
            b_t = kp.tile([128, 1], f32)
            nc.sync.dma_start(lt_t[:], ltm[:])
            nc.sync.dma_start(b_t[:], b_rep[:])

            with tc.tile_pool(name="scan", bufs=1) as sp:
                wa = sp.tile([128, MARG + FD], f32)
                wb = sp.tile([128, MARG + FD], f32)
                off_sb = sp.tile([128, 1], f32)
                nc.vector.memset(wa[:], 0.0)
                nc.vector.memset(wb[:], 0.0)

                # ---- phase A: stream filtered into the scan tile ----
                with tc.tile_pool(name="pa", bufs=1) as pa:
                    w_t = pa.tile([128, TCH * K], f32)
                    nc.scalar.dma_start(w_t[:], w_full[:])
                    for c in range(8):
                        c0 = c * TCH
                        at = pa.tile([128, TCH * K], f32)
                        rt = pa.tile([128, TCH], f32)
                        sg = pa.tile([128, TCH], f32)
                        lg = pa.tile([128, TCH], f32)
                        nc.scalar.dma_start(at[:], attrs_t[:, c0 * K:(c0 + TCH) * K])
                        nc.scalar.dma_start(rt[:], res_t[:, c0:c0 + TCH])
                        nc.scalar.dma_start(sg[:], sgn_t[:, c0:c0 + TCH])
                        nc.vector.tensor_mul(at[:], at[:], w_t[:])
                        nc.vector.reduce_sum(
                            lg[:].rearrange("p (n o) -> p n o", o=1),
                            at[:].rearrange("p (n k) -> p n k", k=K),
                            axis=mybir.AxisListType.X,
                        )
                        nc.scalar.activation(
                            lg[:], lg[:], mybir.ActivationFunctionType.Sigmoid,
                            bias=b_t[:], scale=1.0,
                        )
                        nc.vector.tensor_mul(lg[:], lg[:], rt[:])
                        nc.vector.tensor_mul(
                            wa[:, MARG + c0:MARG + c0 + TCH], lg[:], sg[:]
                        )

                # ---- phase D: prefix sum ----
                cur, nxt = wa, wb
                for s in SHIFTS:
                    nc.vector.tensor_add(
                        nxt[:, MARG:], cur[:, MARG:], cur[:, MARG - s:MARG - s + FD]
                    )
                    cur, nxt = nxt, cur
                with tc.tile_pool(name="psc", bufs=1, space="PSUM") as pp:
                    ps = pp.tile([128, 1], f32, space="PSUM")
                    nc.tensor.matmul(
                        ps[:], lhsT=lt_t[:], rhs=cur[:, MARG + FD - 1:MARG + FD],
                        start=True, stop=True,
                    )
                    nc.vector.tensor_copy(off_sb[:], ps[:])
                nc.vector.tensor_scalar_add(cur[:, MARG:], cur[:, MARG:], off_sb[:])
                nc.sync.dma_start(
                    cums_d[0:DSZ].rearrange("(p f) -> p f", p=128), cur[:, MARG:]
                )

            # ---- phase F: pixel gather (element-wise batched, pipelined) ----
            cg = cums_d[:].rearrange("(n o) -> n o", o=1)
            with tc.tile_pool(name="pf", bufs=1) as pf:
                ix_t = pf.tile([128, NCH_P * CHC], i32)
                yt = pf.tile([128, CH], f32)
                nc.sync.dma_start(ix_t[:], pix_idx[:])
                for j in range(NCH_P):
                    row = j % 128
                    nc.gpsimd.indirect_dma_start(
                        out=yt[row:row + 1, :].rearrange("p (l u) -> p l u", u=1),
                        out_offset=None,
                        in_=cg,
                        in_offset=bass.IndirectOffsetOnAxis(
                            ap=ix_t[:, j * CHC:(j + 1) * CHC], axis=0),
                    )
                    nc.sync.dma_start(
                        y_sh[j:j + 1, :].rearrange("o (t l) -> o t l", t=2),
                        yt[row:row + 1, :].rearrange("p (t l) -> p t l", t=2),
                    )

    _split_excess_waits(nc)
    _cache["nc"] = nc
    return nc


def kernel(weight, bias, residues, attrs2d, tpre, tpost, node_of_pixel,
           numRows, numCols, _profile=[None]):
    weight = np.asarray(weight, np.float32)
    bias = np.asarray(bias, np.float32)
    residues = np.asarray(residues, np.float32)
    attrs2d = np.asarray(attrs2d, np.float32)
    tpre = np.asarray(tpre, np.int64)
    tpost = np.asarray(tpost, np.int64)
    nop = np.asarray(node_of_pixel, np.int64)
    numRows = int(numRows)
    numCols = int(numCols)

    # --- host-side marshalling: order (node, sign) pairs by Euler time t ---
    ordr = np.zeros(DSZ, np.int64)
    sgn = np.zeros(DSZ, np.float32)
    ar = np.arange(N)
    ordr[tpre] = ar
    sgn[tpre] = 1.0
    ordr[tpost] = ar
    sgn[tpost] = -1.0
    attrs_tt = attrs2d[ordr].astype(np.float32)      # (DSZ, K); pads -> row 0
    attrs_tt[sgn == 0.0] = 0.0
    res_tt = residues[ordr].astype(np.float32)
    res_tt[sgn == 0.0] = 0.0
    attrs_w = attrs_tt.reshape(128, FD * K)
    res_w = res_tt.reshape(128, FD)
    sgn_w = sgn.reshape(128, FD)

    w_full = np.tile(weight[None, :], (128, TCH)).astype(np.float32)
    b_rep = np.full((128, 1), np.float32(bias[0]), np.float32)
    lt = (np.arange(128)[:, None] < np.arange(128)[None, :]).astype(np.float32)

    # pixel gather index: t of the pixel's node, wrap transform for the
    # partition-fastest offset consumption order
    t_pix = tpre[nop].astype(np.int32)
    slot_sh = t_pix.reshape(NCORES, NCH_P, CHC, 128).transpose(0, 3, 1, 2) \
        .reshape(NCORES, 128, NCH_P * CHC)

    in_maps = []
    for r in range(NCORES):
        in_maps.append({
            "w_full": w_full,
            "b_rep": b_rep,
            "ltm": lt,
            "attrs_t": attrs_w,
            "res_t": res_w,
            "sgn_t": sgn_w,
            "pix_idx": np.ascontiguousarray(slot_sh[r]),
        })

    nc = _build_program()
    res = run_bass_kernel_spmd(nc, in_maps, list(range(NCORES)),
                               trace=bool(_profile[0]))
    _last_res[0] = res
    if _profile[0] is not None:
        _profile[0] = res.exec_time_ns

    y = np.concatenate([res.results[r]["y_sh"].reshape(-1) for r in range(NCORES)])
    return y.reshape(numRows, numCols).astype(np.float32)


# revision 16
# speedup vs baseline: 1.4351x; 1.0039x over previous
"""Trainium2 Bass kernel for ConnectedFilterLayerWithImplicitJacobian.

Host marshalling orders the 2N (node, +/-) pairs by Euler-tour time t, so the
delta array IS phase A's output in layout order — no scatter, no transpose,
no collectives:

  A. stream filtered = sign * residues * sigmoid(attrs @ w + b) for all 2N
     t-ordered rows directly into the scan tile (replicated on all 8 cores)
  D. prefix-sum (shift-add cascade on DVE + cross-partition fix via PE)
  F. y[p] = cums[tpre[node_of_pixel[p]]] (host-composed index), sharded 1/8:
     128 batched element-wise indirect gathers (8192 descriptors each)
"""
import contextlib
import ctypes
import os
import sys
import types

sys.path.insert(0, "/opt/trn_rl_repo")

import numpy as np

# ---------------------------------------------------------------- shims ----
_SO_PATH = "/opt/axon/libaxon_pjrt.so"


def _install_ntff_shim():
    if "antenv.axon_hooks" in sys.modules:
        return
    try:
        lib = ctypes.CDLL(_SO_PATH)
        ok = hasattr(lib, "axon_start_nrt_profile")
    except OSError:
        ok = False
    if ok:
        lib.axon_start_nrt_profile.argtypes = [ctypes.POINTER(ctypes.c_int64), ctypes.c_size_t]
        lib.axon_start_nrt_profile.restype = ctypes.c_int64
        lib.axon_stop_nrt_profile.argtypes = [ctypes.c_char_p]
        lib.axon_stop_nrt_profile.restype = ctypes.c_int64

        @contextlib.contextmanager
        def _hook(output_dir, device_ids):
            import jax

            jax.devices()
            if device_ids:
                ids = (ctypes.c_int64 * len(device_ids))(*device_ids)
                rc = lib.axon_start_nrt_profile(ids, len(device_ids))
            else:
                rc = lib.axon_start_nrt_profile(None, 0)
            if rc != 0:
                raise RuntimeError(f"axon_start_nrt_profile rc={rc}")
            try:
                yield
            finally:
                n = lib.axon_stop_nrt_profile(str(output_dir).encode())
                if n < 0:
                    raise RuntimeError(f"axon_stop_nrt_profile rc={n}")
    else:
        _hook = None
    mod = types.ModuleType("antenv.axon_hooks")
    mod.get_axon_ntff_profile_hook = lambda: _hook
    mod.set_axon_ntff_profile_hook = lambda h: None
    sys.modules["antenv.axon_hooks"] = mod


_install_ntff_shim()

import concourse.bass as bass
import concourse.bass_utils as bass_utils
import concourse.mybir as mybir
import concourse.tile as tile
from concourse.bass_utils import run_bass_kernel_spmd

# walrus birsim on a large program is prohibitively slow; turn it off
_orig_run_command = bass_utils.run_command


def _patched_run_command(argv, **kwargs):
    argv = ["--enable-birsim=false" if a == "--enable-birsim=true" else a for a in argv]
    return _orig_run_command(argv, **kwargs)


bass_utils.run_command = _patched_run_command

MAX_WAITS = 1


def _split_excess_waits(nc):
    """This container's walrus accepts at most one sync-wait per instruction;
    move extra waits onto injected no-ops ahead of the instruction."""
    nid = 0
    for bb in nc.main_func.blocks:
        insts = bb.instructions
        targets = []
        for idx in range(len(insts)):
            ins = insts[idx]
            si = ins.sync_info
            if si is not None and si.on_wait is not None and len(si.on_wait) > MAX_WAITS:
                targets.append(ins.name)
        for name in targets:
            idx = next(i for i in range(len(insts)) if insts[i].name == name)
            ins = insts[idx]
            w = list(ins.sync_info.on_wait)
            excess, keep = w[:-MAX_WAITS], w[-MAX_WAITS:]
            ins.sync_info.on_wait = keep
            pos = idx
            while excess:
                chunk, excess = excess[:MAX_WAITS], excess[MAX_WAITS:]
                nop = mybir.InstNoOp(
                    name=f"I-ws-{nid}", engine=ins.engine, ins=[], outs=[],
                    sync_info=mybir.SyncInfo(on_wait=chunk, on_update=[]),
                )
                nid += 1
                insts.insert(pos, nop)
                pos += 1


# ------------------------------------------------------------- geometry ----
NCORES = 8
N = 500_000
K = 8
ROWS, COLS = 2048, 4096
P = ROWS * COLS                  # 8388608
PIXN = P // NCORES               # 1048576 pixels per core
T2N = 2 * N                      # 1000000
FD = 7816                        # 128*7816 = 1000448 >= 2N
DSZ = 128 * FD                   # 1000448
DPAD = 128 * 7817
SHIFTS = [1, 2, 4, 8, 16, 32, 64, 128, 256, 512, 1024, 2048, 4096]
MARG = 4096
TCH = 977                        # t-columns per phase-A stream chunk (8*977=FD)
CH = 8192                        # descriptors per batched indirect DMA
CHC = CH // 128                  # 64 offset columns per chunk
NCH_P = PIXN // CH               # 128 pixel chunks

_cache = {}
_last_res = [None]


def _build_program():
    if "nc" in _cache:
        return _cache["nc"]
    nc = bass.Bass()
    f32, i32 = mybir.dt.float32, mybir.dt.int32

    w_full = nc.dram_tensor("w_full", [128, 489 * K], f32, kind="ExternalInput")
    b_rep = nc.dram_tensor("b_rep", [128, 1], f32, kind="ExternalInput")
    ltm = nc.dram_tensor("ltm", [128, 128], f32, kind="ExternalInput")
    attrs_t = nc.dram_tensor("attrs_t", [128, FD * K], f32, kind="ExternalInput")
    res_t = nc.dram_tensor("res_t", [128, FD], f32, kind="ExternalInput")
    sgn_t = nc.dram_tensor("sgn_t", [128, FD], f32, kind="ExternalInput")
    pix_idx = nc.dram_tensor("pix_idx", [128, NCH_P * CHC], i32, kind="ExternalInput")
    y_sh = nc.dram_tensor("y_sh", [NCH_P, CH], f32, kind="ExternalOutput")

    cums_d = nc.dram_tensor("cums_d", [DPAD], f32)

    with tile.TileContext(nc) as tc:
        with tc.tile_pool(name="keep", bufs=1) as kp:
            lt_t = kp.tile([128, 128], f32)
            b_t = kp.tile([128, 1], f32)
            nc.sync.dma_start(lt_t[:], ltm[:])
            nc.sync.dma_start(b_t[:], b_rep[:])

            with tc.tile_pool(name="scan", bufs=1) as sp:
                wa = sp.tile([128, MARG + FD], f32)
                wb = sp.tile([128, MARG + FD], f32)
                off_sb = sp.tile([128, 1], f32)
                # body of wa is fully overwritten by phase A; only the
                # shift-in margins must be zero
                nc.vector.memset(wa[:, :MARG], 0.0)
                nc.vector.memset(wb[:, :MARG], 0.0)

                # ---- phase A: stream filtered into the scan tile ----
                # 16 uneven chunks (8x489 + 8x488 = FD), double-buffered
                with tc.tile_pool(name="pw", bufs=1) as pw, \
                     tc.tile_pool(name="pa", bufs=2) as pa:
                    w_t = pw.tile([128, 489 * K], f32)
                    nc.scalar.dma_start(w_t[:], w_full[:])
                    c0 = 0
                    for c in range(16):
                        tch = 489 if c < 8 else 488
                        at = pa.tile([128, 489 * K], f32)
                        rt = pa.tile([128, 489], f32)
                        sg = pa.tile([128, 489], f32)
                        lg = pa.tile([128, 489], f32)
                        nc.scalar.dma_start(
                            at[:, :tch * K], attrs_t[:, c0 * K:(c0 + tch) * K])
                        nc.scalar.dma_start(rt[:, :tch], res_t[:, c0:c0 + tch])
                        nc.scalar.dma_start(sg[:, :tch], sgn_t[:, c0:c0 + tch])
                        nc.vector.tensor_mul(
                            at[:, :tch * K], at[:, :tch * K], w_t[:, :tch * K])
                        nc.vector.reduce_sum(
                            lg[:, :tch].rearrange("p (n o) -> p n o", o=1),
                            at[:, :tch * K].rearrange("p (n k) -> p n k", k=K),
                            axis=mybir.AxisListType.X,
                        )
                        nc.scalar.activation(
                            lg[:, :tch], lg[:, :tch],
                            mybir.ActivationFunctionType.Sigmoid,
                            bias=b_t[:], scale=1.0,
                        )
                        nc.vector.tensor_mul(lg[:, :tch], lg[:, :tch], rt[:, :tch])
                        nc.vector.tensor_mul(
                            wa[:, MARG + c0:MARG + c0 + tch], lg[:, :tch],
                            sg[:, :tch]
                        )
                        c0 += tch

                # ---- phase D: prefix sum ----
                cur, nxt = wa, wb
                for s in SHIFTS:
                    nc.vector.tensor_add(
                        nxt[:, MARG:], cur[:, MARG:], cur[:, MARG - s:MARG - s + FD]
                    )
                    cur, nxt = nxt, cur
                with tc.tile_pool(name="psc", bufs=1, space="PSUM") as pp:
                    ps = pp.tile([128, 1], f32, space="PSUM")
                    nc.tensor.matmul(
                        ps[:], lhsT=lt_t[:], rhs=cur[:, MARG + FD - 1:MARG + FD],
                        start=True, stop=True,
                    )
                    nc.vector.tensor_copy(off_sb[:], ps[:])
                nc.vector.tensor_scalar_add(cur[:, MARG:], cur[:, MARG:], off_sb[:])
                nc.sync.dma_start(
                    cums_d[0:DSZ].rearrange("(p f) -> p f", p=128), cur[:, MARG:]
                )

            # ---- phase F: pixel gather (element-wise batched, pipelined) ----
            cg = cums_d[:].rearrange("(n o) -> n o", o=1)
            with tc.tile_pool(name="pf", bufs=1) as pf:
                ix_t = pf.tile([128, NCH_P * CHC], i32)
                yt = pf.tile([128, CH], f32)
                nc.sync.dma_start(ix_t[:], pix_idx[:])
                for j in range(NCH_P):
                    row = j % 128
                    nc.gpsimd.indirect_dma_start(
                        out=yt[row:row + 1, :].rearrange("p (l u) -> p l u", u=1),
                        out_offset=None,
                        in_=cg,
                        in_offset=bass.IndirectOffsetOnAxis(
                            ap=ix_t[:, j * CHC:(j + 1) * CHC], axis=0),
                    )
                    nc.sync.dma_start(
                        y_sh[j:j + 1, :].rearrange("o (t l) -> o t l", t=2),
                        yt[row:row + 1, :].rearrange("p (t l) -> p t l", t=2),
                    )

    _split_excess_waits(nc)
    _cache["nc"] = nc
    return nc


def kernel(weight, bias, residues, attrs2d, tpre, tpost, node_of_pixel,
           numRows, numCols, _profile=[None]):
    weight = np.asarray(weight, np.float32)
    bias = np.asarray(bias, np.float32)
    residues = np.asarray(residues, np.float32)
    attrs2d = np.asarray(attrs2d, np.float32)
    tpre = np.asarray(tpre, np.int64)
    tpost = np.asarray(tpost, np.int64)
    nop = np.asarray(node_of_pixel, np.int64)
    numRows = int(numRows)
    numCols = int(numCols)

    # --- host-side marshalling: order (node, sign) pairs by Euler time t ---
    ordr = np.zeros(DSZ, np.int64)
    sgn = np.zeros(DSZ, np.float32)
    ar = np.arange(N)
    ordr[tpre] = ar
    sgn[tpre] = 1.0
    ordr[tpost] = ar
    sgn[tpost] = -1.0
    attrs_tt = attrs2d[ordr].astype(np.float32)      # (DSZ, K); pads -> row 0
    attrs_tt[sgn == 0.0] = 0.0
    res_tt = residues[ordr].astype(np.float32)
    res_tt[sgn == 0.0] = 0.0
    attrs_w = attrs_tt.reshape(128, FD * K)
    res_w = res_tt.reshape(128, FD)
    sgn_w = sgn.reshape(128, FD)

    w_full = np.tile(weight[None, :], (128, 489)).astype(np.float32)
    b_rep = np.full((128, 1), np.float32(bias[0]), np.float32)
    lt = (np.arange(128)[:, None] < np.arange(128)[None, :]).astype(np.float32)

    # pixel gather index: t of the pixel's node, wrap transform for the
    # partition-fastest offset consumption order
    t_pix = tpre[nop].astype(np.int32)
    slot_sh = t_pix.reshape(NCORES, NCH_P, CHC, 128).transpose(0, 3, 1, 2) \
        .reshape(NCORES, 128, NCH_P * CHC)

    in_maps = []
    for r in range(NCORES):
        in_maps.append({
            "w_full": w_full,
            "b_rep": b_rep,
            "ltm": lt,
            "attrs_t": attrs_w,
            "res_t": res_w,
            "sgn_t": sgn_w,
            "pix_idx": np.ascontiguousarray(slot_sh[r]),
        })

    nc = _build_program()
    res = run_bass_kernel_spmd(nc, in_maps, list(range(NCORES)),
                               trace=bool(_profile[0]))
    _last_res[0] = res
    if _profile[0] is not None:
        _profile[0] = res.exec_time_ns

    y = np.concatenate([res.results[r]["y_sh"].reshape(-1) for r in range(NCORES)])
    return y.reshape(numRows, numCols).astype(np.float32)
